# revision 1
# baseline (speedup 1.0000x reference)
"""Trainium2 Bass kernel for nn_GCDDLayer (Gaussian-curvature diffusion layer).

Math (per 512x512 image, zero-padded 3x3 convs):
    ux  = conv(u, SOBEL_X);  uy  = conv(u, SOBEL_Y)
    uxx = conv(ux, SOBEL_X); uxy = conv(ux, SOBEL_Y); uyy = conv(uy, SOBEL_Y)
    G   = (uxx*uyy - uxy^2) / ((1 + ux^2 + uy^2)^2 + 1e-6)
    phi = exp(-|G|); P = phi*ux; Q = phi*uy
    out = u + conv(P, SOBEL_X) + conv(Q, SOBEL_Y)

Strategy: pure data parallel over batch (16 samples -> 8 cores x 2 samples),
each core processes 6 independent 512x512 images (2 samples x 3 channels).
Each image is cut into 5 overlapping 128-row tiles (stride 122; 3-row halo
absorbs the 3-deep conv chain) so every tile flows through the pipeline with
no cross-tile deps. All convolutions run on the TensorEngine as banded-matrix
matmuls (y-direction via the band, x-direction via shifted column access
patterns on zero-padded SBUF tiles, accumulated in PSUM). The pointwise chain
is split across ScalarE (squares/abs/ln/exp/evac copies) and VectorE
(2-input ops); 1/q^2 is computed as exp(-2 ln q) on ScalarE because the DVE
reciprocal measures ~5x slower than table-based ACT ops.
"""

import os

import numpy as np

B, C, H, W = 16, 3, 512, 512
N_CORES = 8
IMGS = (B // N_CORES) * C  # 6 images per core
PAD = 3
BLK = W + 2 * PAD  # 518
NT = 5  # row tiles per image
TILE_STARTS = [0, 122, 244, 366, 384]
OUT_ROWS = [(0, 125), (125, 247), (247, 369), (369, 491), (491, 512)]
WIDTH = NT * BLK  # 2590
PWIDTH = NT * W  # 2560

_CACHE = {}


def _split_multiwaits(nc):
    """Walrus in this container accepts only one sync-wait per instruction;
    Tile emits multi-wait instructions. Split: for an instruction with k>1
    waits, insert k-1 single-wait NoOps before it on the same engine (engine
    queues are strict FIFO, so sequential waiting is equivalent)."""
    import concourse.mybir as mybir

    ctr = [0]

    def fresh(base):
        ctr[0] += 1
        return f"{base}-wsplit{ctr[0]}"

    for f in nc.m.functions:
        for b in f.blocks:
            changed = False
            newlist = []
            for ins in b.instructions:
                si = ins.sync_info
                if si is not None and len(si.on_wait) > 1:
                    waits = list(si.on_wait)
                    for w in waits[:-1]:
                        newlist.append(
                            mybir.InstNoOp(
                                name=fresh(ins.name),
                                engine=ins.engine,
                                debug=ins.debug,
                                ins=[],
                                outs=[],
                                sync_info=mybir.SyncInfo(on_wait=[w], on_update=[]),
                            )
                        )
                    ins.sync_info = mybir.SyncInfo(
                        on_wait=[waits[-1]], on_update=list(si.on_update)
                    )
                    changed = True
                newlist.append(ins)
            if changed:
                b.instructions = newlist


def _band(c0, c1, c2, n=128):
    # lhsT[k, m] = col[k - m + 1] (k: input row partition, m: output row)
    return (
        np.diag(np.full(n, c1))
        + np.diag(np.full(n - 1, c0), 1)
        + np.diag(np.full(n - 1, c2), -1)
    ).astype(np.float32)


def _bands_np():
    return np.stack(
        [
            _band(1, 2, 1),  # BSp: SOBEL_X col dx=+1
            _band(-1, -2, -1),  # BSm: SOBEL_X col dx=-1
            _band(-1, 0, 1),  # BD : SOBEL_Y col dx=+-1
            _band(-2, 0, 2),  # BD2: SOBEL_Y col dx=0
        ]
    )


def _build():
    import concourse.bass as bass
    import concourse.mybir as mybir
    import concourse.tile as tile

    f32 = mybir.dt.float32
    f32r = mybir.dt.float32r
    AF = mybir.ActivationFunctionType
    ALU = mybir.AluOpType

    nc = bass.Bass()
    u_dram = nc.dram_tensor("u", [IMGS, H, W], f32r, kind="ExternalInput")
    bands_dram = nc.dram_tensor("bands", [4, 128, 128], f32r, kind="ExternalInput")
    out_dram = nc.dram_tensor("out", [IMGS, H, W], f32, kind="ExternalOutput")

    def mmcast(ap):
        return ap

    with tile.TileContext(nc) as tc:
        with (
            tc.tile_pool(name="const", bufs=1) as cpool,
            tc.tile_pool(name="pad", bufs=1) as ppool,
            tc.tile_pool(name="pad2", bufs=2) as ppool2,
            tc.tile_pool(name="upad", bufs=2) as upool,
            tc.tile_pool(name="plain", bufs=1) as spool,
            tc.tile_pool(name="plain2", bufs=2) as spool2,
            tc.tile_pool(name="psum", bufs=1, space="PSUM") as qpool,
            tc.tile_pool(name="psum2", bufs=2, space="PSUM") as qpool2,
        ):
            bsp = cpool.tile([128, 128], f32r, tag="bsp")
            bsm = cpool.tile([128, 128], f32r, tag="bsm")
            bd = cpool.tile([128, 128], f32r, tag="bd")
            bd2 = cpool.tile([128, 128], f32r, tag="bd2")
            nc.sync.dma_start(out=bsp[:], in_=bands_dram[0])
            nc.sync.dma_start(out=bsm[:], in_=bands_dram[1])
            nc.sync.dma_start(out=bd[:], in_=bands_dram[2])
            nc.sync.dma_start(out=bd2[:], in_=bands_dram[3])

            def mm_sx(ps, src, t, start=True, stop=True):
                # conv columns of SOBEL_X: dx=-1 -> BSm, dx=+1 -> BSp
                base = BLK * t + PAD
                for j, (b_, dx) in enumerate(((bsm, -1), (bsp, +1))):
                    nc.tensor.matmul(
                        ps[:],
                        mmcast(b_[:]),
                        mmcast(src[:, base + dx : base + dx + W]),
                        start=(j == 0) and start,
                        stop=(j == 1) and stop,
                    )

            def mm_sy(ps, src, t, start=True, stop=True):
                # conv columns of SOBEL_Y: dx=-1 -> BD, 0 -> BD2, +1 -> BD
                base = BLK * t + PAD
                for j, (b_, dx) in enumerate(((bd, -1), (bd2, 0), (bd, +1))):
                    nc.tensor.matmul(
                        ps[:],
                        mmcast(b_[:]),
                        mmcast(src[:, base + dx : base + dx + W]),
                        start=(j == 0) and start,
                        stop=(j == 2) and stop,
                    )

            import contextlib
            reps = int(os.environ.get("GCDD_REPS", "0"))
            loop_cm = tc.For_i(0, reps) if reps > 1 else contextlib.nullcontext()
            with loop_cm:
              for i in range(IMGS):
                u_pad = upool.tile([128, WIDTH], f32r, tag="u")
                uxuy_pad = ppool2.tile([128, 2 * WIDTH], f32r, tag="uxuy")
                pq_pad = ppool.tile([128, 2 * WIDTH], f32r, tag="pq")
                uxxs = spool.tile([128, PWIDTH], f32, tag="uxxs")
                sqxy = spool2.tile([128, PWIDTH], f32, tag="sqxy")
                ta = spool2.tile([128, PWIDTH], f32, tag="ta")
                tb = spool2.tile([128, PWIDTH], f32, tag="tb")
                tnum = spool2.tile([128, PWIDTH], f32, tag="tnum")
                outs = spool2.tile([128, PWIDTH], f32, tag="outs")

                # zero the x-halo pad columns of every shifted-read tensor
                # (pads are never overwritten afterwards, so only fresh pool
                # slots need it: bufs=2 tags on images 0/1, bufs=1 on image 0)
                fresh2 = (u_pad, uxuy_pad) if i < 2 else ()
                fresh1 = (pq_pad,) if i == 0 else ()
                for t_ in fresh2 + fresh1:
                    v = t_[:].bitcast(f32).rearrange("p (n b) -> p n b", b=BLK)
                    nc.vector.memset(v[:, :, 0:PAD], 0)
                    nc.vector.memset(v[:, :, PAD + W : BLK], 0)

                # 3D center views [128, NT, 512]
                uc = u_pad[:].bitcast(f32).rearrange("p (n b) -> p n b", b=BLK)[:, :, PAD : PAD + W]
                uxuyv = uxuy_pad[:].rearrange("p (m n b) -> p m n b", m=2, b=BLK)
                uxuyc = uxuyv[:, :, :, PAD : PAD + W]
                uxc = uxuy_pad[:, :WIDTH].bitcast(f32).rearrange(
                    "p (n b) -> p n b", b=BLK
                )[:, :, PAD : PAD + W]
                uyc = uxuy_pad[:, WIDTH:].bitcast(f32).rearrange(
                    "p (n b) -> p n b", b=BLK
                )[:, :, PAD : PAD + W]
                pqc = pq_pad[:].rearrange("p (m n b) -> p m n b", m=2, b=BLK)[
                    :, :, :, PAD : PAD + W
                ]
                uxx3 = uxxs[:].rearrange("p (n b) -> p n b", b=W)
                sqxy3 = sqxy[:].rearrange("p (n b) -> p n b", b=W)
                ta3 = ta[:].rearrange("p (n b) -> p n b", b=W)
                tb3 = tb[:].rearrange("p (n b) -> p n b", b=W)
                tnum3 = tnum[:].rearrange("p (n b) -> p n b", b=W)
                outs3 = outs[:].rearrange("p (n b) -> p n b", b=W)

                # load u tiles
                for t in range(NT):
                    st = TILE_STARTS[t]
                    nc.sync.dma_start(
                        out=u_pad[:, BLK * t + PAD : BLK * t + PAD + W],
                        in_=u_dram[i, st : st + 128, :],
                    )

                # ---- stage A: first derivatives -------------------------
                for t in range(NT):
                    ps_a = qpool2.tile([128, 2 * W], f32, tag="ps_a")
                    mm_sx(ps_a[:, :W], u_pad, t)
                    mm_sy(ps_a[:, W:], u_pad, t)
                    nc.scalar.copy(
                        uxuyc[:, :, t, :],
                        ps_a[:].rearrange("p (m w) -> p m w", m=2),
                    )

                # ---- stage B: second derivatives ------------------------
                for t in range(NT):
                    ps_uxx = qpool.tile([128, W], f32, tag="ps_uxx")
                    mm_sx(ps_uxx, uxuy_pad[:, :WIDTH], t)
                    nc.scalar.copy(uxx3[:, t, :], ps_uxx[:])
                    ps_uxy = qpool.tile([128, W], f32, tag="ps_uxy")
                    mm_sy(ps_uxy, uxuy_pad[:, :WIDTH], t)
                    nc.scalar.square(sqxy3[:, t, :], ps_uxy[:])
                    ps_uyy = qpool.tile([128, W], f32, tag="ps_uyy")
                    mm_sy(ps_uyy, uxuy_pad[:, WIDTH:], t)
                    # nm = uxx * uyy (one PSUM operand max per DVE op)
                    nc.vector.tensor_mul(tnum3[:, t, :], ps_uyy[:], uxx3[:, t, :])

                # ---- pointwise chain (chunked so DVE/ACT pipeline) ------
                import json as _json
                _ck = _json.loads(os.environ.get("GCDD_CHUNKS", "[[0,1],[1,3],[3,5]]"))
                for lo, hi in _ck:
                    s = (slice(None), slice(lo, hi), slice(None))
                    nc.scalar.square(ta3[s], uxc[s])  # ta = ux^2 (ACT, balance)
                    nc.vector.tensor_mul(tb3[s], uyc[s], uyc[s])  # tb = uy^2
                    nc.vector.scalar_tensor_tensor(  # ta = q = (ux^2+1) + uy^2
                        ta3[s], ta3[s], 1.0, tb3[s], ALU.add, ALU.add
                    )
                    # 1/q^2 = exp(-2 ln q): DVE reciprocal measures ~5x slower
                    # than ACT table ops; Ln/Exp share one ACT table set.
                    nc.scalar.activation(ta3[s], ta3[s], AF.Ln)
                    nc.scalar.activation(tb3[s], ta3[s], AF.Exp, scale=-2.0)
                    nc.vector.tensor_sub(  # num = uxx*uyy - uxy^2
                        tnum3[s], tnum3[s], sqxy3[s]
                    )
                    nc.scalar.activation(tnum3[s], tnum3[s], AF.Abs)  # |num|
                    nc.vector.tensor_mul(tnum3[s], tnum3[s], tb3[s])  # aG
                    nc.scalar.activation(  # phi = exp(-aG)
                        tnum3[s], tnum3[s], AF.Exp, scale=-1.0
                    )
                    # P|Q = phi * (ux|uy) in one op (phi broadcast over m)
                    sm = (slice(None), slice(None), slice(lo, hi), slice(None))
                    nc.vector.tensor_mul(
                        pqc[sm],
                        tnum3[s].unsqueeze(1).broadcast_to((128, 2, hi - lo, W)),
                        uxuyc[sm].bitcast(f32),
                    )

                # ---- stage C: divergence + residual ---------------------
                for t in range(NT):
                    ps_div = qpool.tile([128, W], f32, tag="ps_div")
                    mm_sx(ps_div, pq_pad[:, :WIDTH], t, start=True, stop=False)
                    mm_sy(ps_div, pq_pad[:, WIDTH:], t, start=False, stop=True)
                    nc.vector.tensor_add(outs3[:, t, :], ps_div[:], uc[:, t, :])
                    st = TILE_STARTS[t]
                    lo, hi = OUT_ROWS[t]
                    nc.sync.dma_start(
                        out=out_dram[i, lo:hi, :],
                        in_=outs[lo - st : hi - st, W * t : W * t + W],
                    )

    _split_multiwaits(nc)
    return nc


def _get_nc():
    if "nc" not in _CACHE:
        _CACHE["nc"] = _build()
    return _CACHE["nc"]


def kernel(u: np.ndarray, theta: np.ndarray = None) -> np.ndarray:
    from concourse.bass_utils import run_bass_kernel_spmd

    nc = _get_nc()
    u = np.ascontiguousarray(u, dtype=np.float32)
    bands = np.ascontiguousarray(_bands_np())
    per = B // N_CORES
    in_maps = [
        {
            "u": u[i * per : (i + 1) * per].reshape(IMGS, H, W),
            "bands": bands,
        }
        for i in range(N_CORES)
    ]
    res = run_bass_kernel_spmd(
        nc,
        in_maps,
        core_ids=list(range(N_CORES)),
        trace=os.environ.get("GCDD_TRACE", "0") == "1",
    )
    _CACHE["last_result"] = res
    out = np.empty((B, C, H, W), np.float32)
    for i in range(N_CORES):
        out[i * per : (i + 1) * per] = res.results[i]["out"].reshape(per, C, H, W)
    return out

